# revision 2
# baseline (speedup 1.0000x reference)
"""Trainium2 Bass kernel for EnhancedFastKANLayer.

Reference computation (B=16384, D=O=512, G=8 grids):
    x_norm = (x - mean) * rsqrt(var + eps) * gamma + beta          # BN inference
    basis[b,d,g] = exp(-((x_norm[b,d] - grid[g]) / denom)^2)       # RBF expansion
    out = basis.reshape(B, D*G) @ W_spline + b_spline
        + relu(x) @ W_base + b_base + x

Strategy:
  - Data parallel: batch 16384 sharded 8 ways (2048 rows/core); weights
    replicated. No collectives.
  - All on-chip compute happens in the transposed layout [feature, batch]:
    the output is produced as out_T [O, B_shard] and transposed back on the
    host. This makes BN/basis per-partition-scalar ops, lets the spline
    matmul consume basis tiles directly as the moving operand, and makes the
    residual (+x) a single extra identity matmul into the same PSUM
    accumulator.
  - Factored RBF: with u = x_norm/denom, c_g = grid_g/denom:
        basis_g = exp(-(u-c_g)^2) = exp(-u^2) * exp(2*c_g*u - c_g^2)
    so per d-tile we pay 2 ACT ops for A=exp(-u^2) once, then 1 ACT exp +
    1 DVE mul per grid instead of 2 ACT ops per grid.
  - u is clamped to [-11, 11] so exp(2*c_g*u - c_g^2) <= e^72 never
    overflows fp32/bf16 (|u| <= ~11.4 for this input distribution, and for
    |u| > 11 every basis value is < e^-49, i.e. exactly 0 in the output).
  - x is pre-cast to fp16 on the host: DMA-transpose (xbar) only supports
    16-bit dtypes, and fp16 keeps the BN input error at 2^-11 relative.
  - W_spline is pre-reordered on the host to K-order (dt, g, d_in) matching
    the order basis tiles are produced on chip, cast to bf16.
  - Matmul: out_T[o_sub, b] accumulates 37 matmuls per PSUM tile:
    32 spline K-chunks + 4 relu(x)@W_base K-chunks + 1 identity*xT residual.
"""

import numpy as np
import ml_dtypes
from contextlib import ExitStack

import concourse.bass as bass
import concourse.tile as tile
from concourse import bacc, mybir
from concourse._compat import with_exitstack
from concourse.bass_utils import run_bass_kernel_spmd
from concourse.masks import make_identity

N_CORES = 8
BATCH, IN_DIM, OUT_DIM, G = 16384, 512, 512, 8
B_SHARD = BATCH // N_CORES          # 2048
B_CHUNK = 1024                      # batch columns processed per chunk
GRID_MIN, GRID_MAX, BN_EPS = -2.0, 2.0, 1e-3
DENOM = (GRID_MAX - GRID_MIN) / G   # 0.5
UCLAMP = 11.0
N_DT = IN_DIM // 128                # 4 d-tiles
K_SPLINE = N_DT * G                 # 32 spline K-chunks
K_BASE = N_DT                       # 4 base K-chunks
N_OSUB = OUT_DIM // 128             # 4 output partition tiles

F32 = mybir.dt.float32
F16 = mybir.dt.float16
BF16 = mybir.dt.bfloat16


def _grid_consts():
    grid = np.linspace(GRID_MIN, GRID_MAX, G, dtype=np.float32)
    c = (grid / np.float32(DENOM)).astype(np.float32)        # grid in u-units
    return c


def _col(vec_ap, start, p=128):
    """View rows [start, start+p) of a 1-D DRAM tensor as a [p, 1] AP."""
    return bass.AP(
        tensor=vec_ap.tensor,
        offset=vec_ap.offset + start,
        ap=[[1, p], [0, 1]],
    )


@with_exitstack
def _body(ctx, tc, x16, w_sp, w_b, uscale, ushift, bias_o, out_t,
          b_shard, b_chunk):
    nc = tc.nc
    n_chunks = b_shard // b_chunk
    n_bh = b_chunk // 512            # 512-wide moving-operand slices
    c = _grid_consts()
    k_total = K_SPLINE + K_BASE + 1  # + residual identity matmul

    const_pool = ctx.enter_context(tc.tile_pool(name="const", bufs=1))
    w_pool = ctx.enter_context(tc.tile_pool(name="w", bufs=1))
    xt_pool = ctx.enter_context(tc.tile_pool(name="xt", bufs=2 * N_DT))
    u_pool = ctx.enter_context(tc.tile_pool(name="u", bufs=2))
    sq_pool = ctx.enter_context(tc.tile_pool(name="sq", bufs=2))
    a_pool = ctx.enter_context(tc.tile_pool(name="a", bufs=2))
    e_pool = ctx.enter_context(tc.tile_pool(name="e", bufs=3))
    # spline basis tiles: 32 per chunk stay resident through the chunk's
    # matmul phase; extra slots let the next chunk's production run ahead.
    basis_pool = ctx.enter_context(tc.tile_pool(name="basis", bufs=K_SPLINE + 8))
    relu_pool = ctx.enter_context(tc.tile_pool(name="relu", bufs=2 * N_DT))
    psum_pool = ctx.enter_context(
        tc.tile_pool(name="psum", bufs=8, space="PSUM"))
    out_pool = ctx.enter_context(tc.tile_pool(name="outs", bufs=4))

    # ---- constants / weights (loaded once) ----
    w_tile = w_pool.tile([128, K_SPLINE, OUT_DIM], F16)
    nc.sync.dma_start(out=w_tile, in_=w_sp)
    wb_tile = w_pool.tile([128, K_BASE, OUT_DIM], F16)
    nc.sync.dma_start(out=wb_tile, in_=w_b)

    uscale_sb = const_pool.tile([128, N_DT], F32)
    ushift_sb = const_pool.tile([128, N_DT], F32)
    bias_sb = const_pool.tile([128, N_OSUB], F32)
    for dt in range(N_DT):
        nc.sync.dma_start(out=uscale_sb[:, dt:dt + 1], in_=_col(uscale, dt * 128))
        nc.sync.dma_start(out=ushift_sb[:, dt:dt + 1], in_=_col(ushift, dt * 128))
    for osub in range(N_OSUB):
        nc.sync.dma_start(out=bias_sb[:, osub:osub + 1], in_=_col(bias_o, osub * 128))
    negcsq = const_pool.tile([128, G], F32)
    for g in range(G):
        nc.vector.memset(negcsq[:, g:g + 1], -float(c[g]) ** 2)
    ident = const_pool.tile([128, 128], F16)
    make_identity(nc, ident)

    for ch in range(n_chunks):
        b0 = ch * b_chunk
        # ---- producer phase: transposed x, BN, relu, basis ----
        xts = []
        relus = []
        basis = []
        for dt in range(N_DT):
            xt = xt_pool.tile([128, b_chunk], F16, tag="xt")
            nc.sync.dma_start(
                out=xt,
                in_=x16[b0:b0 + b_chunk, dt * 128:(dt + 1) * 128],
                transpose=True,
            )
            xts.append(xt)
            u = u_pool.tile([128, b_chunk], F32, tag="u")
            nc.vector.tensor_scalar(
                out=u, in0=xt,
                scalar1=uscale_sb[:, dt:dt + 1], scalar2=ushift_sb[:, dt:dt + 1],
                op0=mybir.AluOpType.mult, op1=mybir.AluOpType.add,
            )
            nc.vector.tensor_scalar(
                out=u, in0=u, scalar1=-UCLAMP, scalar2=UCLAMP,
                op0=mybir.AluOpType.max, op1=mybir.AluOpType.min,
            )
            rl = relu_pool.tile([128, b_chunk], F16, tag="relu")
            nc.vector.tensor_scalar_max(out=rl, in0=xt, scalar1=0.0)
            relus.append(rl)
            sq = sq_pool.tile([128, b_chunk], F32, tag="sq")
            nc.scalar.activation(out=sq, in_=u,
                                 func=mybir.ActivationFunctionType.Square)
            a_t = a_pool.tile([128, b_chunk], F32, tag="a")
            nc.scalar.activation(out=a_t, in_=sq,
                                 func=mybir.ActivationFunctionType.Exp,
                                 scale=-1.0)
            for g in range(G):
                e_t = e_pool.tile([128, b_chunk], F32, tag="e")
                nc.scalar.activation(
                    out=e_t, in_=u, func=mybir.ActivationFunctionType.Exp,
                    bias=negcsq[:, g:g + 1], scale=float(2.0 * c[g]),
                )
                bt = basis_pool.tile([128, b_chunk], F16, tag="basis")
                nc.vector.tensor_mul(out=bt, in0=a_t, in1=e_t)
                basis.append(bt)

        # ---- matmul + epilogue phase ----
        for osub in range(N_OSUB):
            for bh in range(n_bh):
                ps = psum_pool.tile([128, 512], F32, tag="ps")
                bsl = slice(bh * 512, (bh + 1) * 512)
                kc = 0
                for i in range(K_SPLINE):
                    nc.tensor.matmul(
                        ps, lhsT=w_tile[:, i, osub * 128:(osub + 1) * 128],
                        rhs=basis[i][:, bsl],
                        start=(kc == 0), stop=(kc == k_total - 1))
                    kc += 1
                for dt in range(N_DT):
                    nc.tensor.matmul(
                        ps, lhsT=wb_tile[:, dt, osub * 128:(osub + 1) * 128],
                        rhs=relus[dt][:, bsl],
                        start=(kc == 0), stop=(kc == k_total - 1))
                    kc += 1
                # residual: out_T[osub] += I . xT[osub]
                nc.tensor.matmul(
                    ps, lhsT=ident, rhs=xts[osub][:, bsl],
                    start=(kc == 0), stop=(kc == k_total - 1))
                kc += 1
                ot = out_pool.tile([128, 512], F32, tag="ot")
                nc.vector.tensor_scalar_add(
                    out=ot, in0=ps, scalar1=bias_sb[:, osub:osub + 1])
                nc.sync.dma_start(
                    out=out_t[osub * 128:(osub + 1) * 128,
                              b0 + bh * 512:b0 + (bh + 1) * 512],
                    in_=ot)


def build_program(b_shard=B_SHARD, b_chunk=B_CHUNK):
    nc = bacc.Bacc("TRN2", target_bir_lowering=False, debug=False,
                   num_devices=N_CORES)
    x16 = nc.dram_tensor("x16", [b_shard, IN_DIM], F16,
                         kind="ExternalInput").ap()
    w_sp = nc.dram_tensor("w_sp", [128, K_SPLINE, OUT_DIM], F16,
                          kind="ExternalInput").ap()
    w_b = nc.dram_tensor("w_base", [128, K_BASE, OUT_DIM], F16,
                         kind="ExternalInput").ap()
    uscale = nc.dram_tensor("uscale", [IN_DIM], F32, kind="ExternalInput").ap()
    ushift = nc.dram_tensor("ushift", [IN_DIM], F32, kind="ExternalInput").ap()
    bias_o = nc.dram_tensor("bias_o", [OUT_DIM], F32, kind="ExternalInput").ap()
    out_t = nc.dram_tensor("out_t", [OUT_DIM, b_shard], F32,
                           kind="ExternalOutput").ap()
    with tile.TileContext(nc) as tc:
        _body(tc, x16, w_sp, w_b, uscale, ushift, bias_o, out_t,
              b_shard, b_chunk)
    nc.compile()
    return nc


def make_in_maps(x, gamma, beta, moving_mean, moving_var, W_spline, b_spline,
                 W_base, b_base, n_cores=N_CORES):
    """Host-side preprocessing + per-core input shards."""
    x = np.asarray(x, dtype=np.float32)
    gamma = np.asarray(gamma, dtype=np.float32)
    beta = np.asarray(beta, dtype=np.float32)
    moving_mean = np.asarray(moving_mean, dtype=np.float32)
    moving_var = np.asarray(moving_var, dtype=np.float32)
    W_spline = np.asarray(W_spline, dtype=np.float32)
    W_base = np.asarray(W_base, dtype=np.float32)
    b_spline = np.asarray(b_spline, dtype=np.float32)
    b_base = np.asarray(b_base, dtype=np.float32)

    scale = gamma / np.sqrt(moving_var + np.float32(BN_EPS))
    shift = beta - moving_mean * scale
    uscale = (scale / np.float32(DENOM)).astype(np.float32)
    ushift = (shift / np.float32(DENOM)).astype(np.float32)

    x16 = x.astype(np.float16)
    # K-order on chip is (dt, g, d_in): kc = dt*8+g covers d in
    # [dt*128, (dt+1)*128) at grid g.  W_spline rows are (d, g)-ordered.
    w_r = (W_spline.reshape(N_DT, 128, G, OUT_DIM)
           .transpose(0, 2, 1, 3)            # (dt, g, d_in, o)
           .reshape(K_SPLINE, 128, OUT_DIM)
           .transpose(1, 0, 2))              # (d_in, kc, o)
    w_sp = np.ascontiguousarray(w_r).astype(np.float16)
    w_b = np.ascontiguousarray(
        W_base.reshape(K_BASE, 128, OUT_DIM).transpose(1, 0, 2)
    ).astype(np.float16)
    bias_o = (b_spline + b_base).astype(np.float32)

    b_shard = x.shape[0] // n_cores
    return [
        {
            "x16": x16[ci * b_shard:(ci + 1) * b_shard],
            "w_sp": w_sp,
            "w_base": w_b,
            "uscale": uscale,
            "ushift": ushift,
            "bias_o": bias_o,
        }
        for ci in range(n_cores)
    ]


_PROGRAM = None


def kernel(x, gamma, beta, moving_mean, moving_var, W_spline, b_spline,
           W_base, b_base):
    global _PROGRAM
    if _PROGRAM is None:
        _PROGRAM = build_program()
    in_maps = make_in_maps(x, gamma, beta, moving_mean, moving_var,
                           W_spline, b_spline, W_base, b_base)
    res = run_bass_kernel_spmd(_PROGRAM, in_maps, core_ids=list(range(N_CORES)))
    out = np.concatenate(
        [np.ascontiguousarray(res.results[ci]["out_t"].T)
         for ci in range(N_CORES)], axis=0)
    return out.astype(np.float32)
